# revision 37
# baseline (speedup 1.0000x reference)
"""Trainium2 Bass kernel for nn_AttentionNet (conv -> BiLSTM -> MHA -> readout).

Data parallel over batch: 16 samples / 8 cores = 2 per core. All weights
replicated. Global mean/std readout uses an on-device AllReduce of
[sum(r), sum(r^2)].

Self-contained: hardcodes all shapes; imports concourse from /opt/trn_rl_repo.
"""
import dataclasses
import os
import sys

sys.path.insert(0, '/opt/trn_rl_repo')

import numpy as np
import ml_dtypes
import orjson

import concourse.bass as bass
import concourse.tile as tile
from concourse import mybir
import concourse.bass_utils as bass_utils
import concourse.bass2jax as bass2jax
from concourse.bass_utils import run_bass_kernel_spmd

# ---------------------------------------------------------------- constants
N_CORES = 8
B = 16
BL = B // N_CORES          # 2 samples per core
FILT = 13
NK = 200                   # conv out channels
POOL = 6
LP_IN, LE_IN = 3000, 2000
LP, LE = LP_IN - FILT + 1, LE_IN - FILT + 1          # 2988, 1988
SP, SE = LP // POOL, LE // POOL                      # 498, 331
S = SP + SE                                          # 829
H = 100                    # lstm hidden
NH, HD = 8, 32
MH = 100
BN_EPS = 1e-5

F32 = mybir.dt.float32
F32R = mybir.dt.float32r
BF16 = mybir.dt.bfloat16
AF = mybir.ActivationFunctionType
ALU = mybir.AluOpType

KSTEPS = int(os.environ.get("KSTEPS", S))   # debug: limit recurrence steps

# ------------------------------------------------------- walrus wait patch
_NOP_ID = [0]


def _split_multiwait_bir(bir_json: bytes) -> bytes:
    m = orjson.loads(bir_json)
    changed = False
    for f in m["functions"]:
        for b in f["blocks"]:
            out = []
            for ins in b["instructions"]:
                sync = ins.get("sync_info")
                waits = sync.get("on_wait") if sync else None
                if waits and len(waits) > 1:
                    changed = True
                    for w in waits[:-1]:
                        _NOP_ID[0] += 1
                        out.append({
                            "debug": ins.get("debug", 0),
                            "engine": ins["engine"],
                            "ins": [], "outs": [],
                            "name": f"waitsplit-{_NOP_ID[0]}",
                            "opcode": "NoOp",
                            "sync_info": {"on_update": [], "on_wait": [w]},
                        })
                    sync["on_wait"] = [waits[-1]]
                out.append(ins)
            if changed:
                b["instructions"] = out
    return orjson.dumps(m) if changed else bir_json


_orig_cbk = bass_utils.compile_bir_kernel


def _patched_cbk(bir_json, tmpdir, neff_name="file.neff", **kw):
    if isinstance(bir_json, str):
        bir_json = bir_json.encode()
    return _orig_cbk(_split_multiwait_bir(bir_json), tmpdir, neff_name, **kw)


if getattr(bass_utils.compile_bir_kernel, "__name__", "") != "_patched_cbk":
    bass_utils.compile_bir_kernel = _patched_cbk
    bass2jax.compile_bir_kernel = _patched_cbk


# ------------------------------------------------------------- AP helpers
def strided(ap, dim, start, step, count):
    """View of `ap` along dim `dim` with arbitrary (possibly negative) step,
    in units of that dim's existing stride."""
    steps = [s for s, c in ap.ap]
    off = ap.offset + start * steps[dim]
    newap = list(ap.ap)
    newap[dim] = [steps[dim] * step, count]
    return dataclasses.replace(ap, offset=off, ap=newap)


def chunks(total, size):
    out, pos = [], 0
    while pos < total:
        out.append((pos, min(size, total - pos)))
        pos += size
    return out


S_HALVES = [(0, 512), (511, 318)]   # overlap 1 col: f32r needs even N
T_CHUNKS = chunks(S, 128)           # 7 chunks: 6x128 + 61


# =================================================================== build
def build_nc(single_core=False):
    nc = bass.Bass("TRN2", target_bir_lowering=False, debug=False,
                   num_devices=1 if single_core else N_CORES)
    nc._single_core_prof = single_core

    t = {}
    t["xp"] = nc.dram_tensor("xp", [BL, 4, LP_IN], F32R, kind="ExternalInput")
    t["xe"] = nc.dram_tensor("xe", [BL, 4, LE_IN], F32R, kind="ExternalInput")
    t["wconv"] = nc.dram_tensor("wconv", [52, 2 * NK], F32R, kind="ExternalInput")
    t["bconv"] = nc.dram_tensor("bconv", [H, 4], F32, kind="ExternalInput")
    t["wih0"] = nc.dram_tensor("wih0", [100, 1600], F32R, kind="ExternalInput")
    t["wih1"] = nc.dram_tensor("wih1", [100, 1600], BF16, kind="ExternalInput")
    t["blstm"] = nc.dram_tensor("blstm", [100, 16], F32, kind="ExternalInput")
    t["whh"] = nc.dram_tensor("whh", [100, 2048], BF16, kind="ExternalInput")
    # wqk: [qk, kc, k(100), headpair(4), 64]
    t["wqk"] = nc.dram_tensor("wqk", [100, 1024], BF16, kind="ExternalInput")
    t["bqk"] = nc.dram_tensor("bqk", [64, 8], F32, kind="ExternalInput")
    t["wv"] = nc.dram_tensor("wv", [101, 512], BF16, kind="ExternalInput")
    t["wmh"] = nc.dram_tensor("wmh", [32, NH * MH], F32, kind="ExternalInput")
    t["bmh"] = nc.dram_tensor("bmh", [1, MH], F32, kind="ExternalInput")
    t["wfc"] = nc.dram_tensor("wfc", [100, 2], F32, kind="ExternalInput")
    t["cfc"] = nc.dram_tensor("cfc", [1, 4], F32, kind="ExternalInput")
    t["ident"] = nc.dram_tensor("ident", [100, 100], BF16, kind="ExternalInput")
    # cfc cols: [-colsum(fc3_w)[0], -colsum(fc3_w)[1], fc3_b[0], fc3_b[1]]

    t["pattn"] = nc.dram_tensor("pattn", [BL, S, NH * S], F32,
                                kind="ExternalOutput")
    t["probs"] = nc.dram_tensor("probs", [BL, 2], F32, kind="ExternalOutput")

    with tile.TileContext(nc) as tc:
        _kernel_body(nc, tc, t)
    return nc


def _kernel_body(nc, tc, t):
    from contextlib import ExitStack
    with ExitStack() as ctx:
        pw = ctx.enter_context(tc.tile_pool(name="weights", bufs=1))
        pbig = ctx.enter_context(tc.tile_pool(name="big", bufs=1))
        ppers = ctx.enter_context(tc.tile_pool(name="pers", bufs=1))
        psc = ctx.enter_context(tc.tile_pool(name="scratch", bufs=3))
        pim = ctx.enter_context(tc.tile_pool(name="imcol", bufs=2))
        pe_pool = ctx.enter_context(tc.tile_pool(name="etile", bufs=8))
        pvt = ctx.enter_context(tc.tile_pool(name="vtile", bufs=1))
        pst = ctx.enter_context(tc.tile_pool(name="staging", bufs=3))
        pdram = ctx.enter_context(tc.tile_pool(name="dram", bufs=1, space="DRAM"))
        # PSUM banks: ps512(2) + gates/at shared (2x2=4) + ps_big (2) = 8
        pp = ctx.enter_context(tc.tile_pool(name="ps_main", bufs=2, space="PSUM"))
        ppg = ctx.enter_context(tc.tile_pool(name="ps_gates", bufs=2, space="PSUM"))

        # ---------------- load weights ----------------
        wconv = pw.tile([52, 2 * NK], F32R, tag="wconv")
        nc.sync.dma_start(wconv[:], t["wconv"].ap())
        bconv = pw.tile([H, 4], F32, tag="bconv")          # col = br*2 + kc
        nc.sync.dma_start(bconv[:], t["bconv"].ap())
        wih0 = pw.tile([100, 1600], F32R, tag="wih0")  # (dir,kc,gate)-major
        nc.sync.dma_start(wih0[:], t["wih0"].ap())
        wih1 = pw.tile([100, 1600], BF16, tag="wih1")
        nc.sync.dma_start(wih1[:], t["wih1"].ap())
        blstm = pw.tile([100, 16], F32, tag="blstm")       # col = (l*2+d)*4+g
        nc.sync.dma_start(blstm[:], t["blstm"].ap())
        whh = pw.tile([100, 2048], BF16, tag="whh")        # col = (l*8+g*2+d)*128+j
        nc.sync.dma_start(whh[:], t["whh"].ap())
        wqk = pw.tile([100, 1024], BF16, tag="wqk")        # (qk,kc,hp)-major, 64 each
        nc.sync.dma_start(wqk[:], t["wqk"].ap())
        bqk = pw.tile([64, 8], F32, tag="bqk")             # col = qk*4+hp
        nc.sync.dma_start(bqk[:], t["bqk"].ap())
        wv = pw.tile([101, 512], BF16, tag="wv")           # col = (kc*2+hg)*128+j
        nc.sync.dma_start(wv[:], t["wv"].ap())
        wmh = pw.tile([32, NH * MH], F32, tag="wmh")      # col = h*MH+m
        nc.sync.dma_start(wmh[:], t["wmh"].ap())
        bmh = pw.tile([1, MH], F32, tag="bmh")
        nc.sync.dma_start(bmh[:], t["bmh"].ap())
        wfc = pw.tile([100, 2], F32, tag="wfc")
        nc.sync.dma_start(wfc[:], t["wfc"].ap())
        cfc = pw.tile([1, 4], F32, tag="cfc")
        nc.sync.dma_start(cfc[:], t["cfc"].ap())

        ones_row_bf = pw.tile([1, 128], BF16, tag="ones_row_bf")
        nc.vector.memset(ones_row_bf[:], 1.0)
        ones_row_f128 = pw.tile([1, 128], F32, tag="ones_row_f128")
        nc.vector.memset(ones_row_f128[:], 1.0)
        ident = pw.tile([100, 100], BF16, tag="ident")
        nc.sync.dma_start(ident[:], t["ident"].ap())
        ones_col_f = pw.tile([128, 1], F32, tag="ones_col_f")
        nc.vector.memset(ones_col_f[:], 1.0)
        ones_row_f = pw.tile([1, 2], F32, tag="ones_row_f")
        nc.vector.memset(ones_row_f[:], 1.0)

        # bmh replicated across partitions (for the scaled mh accumulation)
        bmhrep = pw.tile([128, MH], F32, tag="bmhrep")
        bmh_ps = pp.tile([128, 512], F32, tag="ps512", name="bmh_ps")
        nc.tensor.matmul(bmh_ps[:, 0:MH], ones_row_f128[:], bmh[:],
                         start=True, stop=True)
        nc.scalar.copy(bmhrep[:], bmh_ps[:, 0:MH])

        # ------------- conv + pool -> xconv[b][kc] [100, S] f32r -------------
        # (xconv shares slots with attention's qk tiles: tags qkx_0..3)
        xconv = [[pvt.tile([100, S], F32R, tag=f"qkx_{2 * b + kc}",
                           name=f"xconv_{b}_{kc}")
                  for kc in range(2)] for b in range(BL)]
        only_dma = os.environ.get("PHASE") == "im2col"
        for b in range(BL):
            for br, (x_d, LOUT, SPOOL, scol) in enumerate(
                    [(t["xp"], LP, SP, 0), (t["xe"], LE, SE, SP)]):
                im = pim.tile([52, LP], F32R, tag="im2col")
                imv = im[:].rearrange("(c k) l -> k c l", k=FILT)
                for k in range(FILT):
                    nc.sync.dma_start(imv[k, :, 0:LOUT], x_d.ap()[b][:, k:k + LOUT])
                for kc in range(2 if not only_dma else 0):
                    for g0, gw in chunks(SPOOL, 85):
                        l0, lw = g0 * POOL, gw * POOL
                        ps = pp.tile([128, 512], F32, tag="ps512")
                        nc.tensor.matmul(
                            ps[0:100, 0:lw],
                            wconv[:, br * 2 * H + kc * H: br * 2 * H + kc * H + H],
                            im[:, l0:l0 + lw], start=True, stop=True)
                        pooled = psc.tile([100, 85], F32, tag="pooled")
                        nc.vector.tensor_reduce(
                            pooled[:, 0:gw],
                            ps[0:100, 0:lw].rearrange("p (g u) -> p g u", u=POOL),
                            axis=mybir.AxisListType.X, op=ALU.max)
                        nc.scalar.activation(
                            xconv[b][kc][:, scol + g0: scol + g0 + gw],
                            pooled[:, 0:gw], AF.Relu,
                            bias=bconv[:, br * 2 + kc: br * 2 + kc + 1])

        # reversed-t copies for the backward direction reads
        xconv_rev = [[pvt.tile([100, S], F32R, tag=f"qkx_{4 + 2 * b + kc}",
                               name=f"xconvr_{b}_{kc}")
                      for kc in range(2)] for b in range(BL)]
        for b in range(BL):
            for kc in range(2):
                nc.sync.dma_start(xconv_rev[b][kc][:],
                                  strided(xconv[b][kc][:], 1, S - 1, -1, S))

        if os.environ.get("PHASE") == "conv":
            return
        # ---------------- LSTM ----------------
        # XG [100, 16*S] f32 : col = 16k + 4g + c ; chain c = 2*dir + b
        xg = pbig.tile([100, 16 * S], BF16, tag="xg")
        # H histories [101, 4*S] bf16 : col = 4k + c ; row 100 = ones
        hh = [[pbig.tile([101, 2 * S], BF16, tag=f"hh_{l}_{dr}",
                         name=f"hh_{l}_{dr}") for dr in range(2)]
              for l in range(2)]
        hhrev = [[pbig.tile([100, 2 * S], BF16, tag=f"hhrev_{l}_{dr}",
                            name=f"hhrev_{l}_{dr}") for dr in range(2)]
                 for l in range(2)]
        ones_4s = pw.tile([1, 2 * S], BF16, tag="ones_4s")
        nc.vector.memset(ones_4s[:], 1.0)
        for l in range(2):
            for dr in range(2):
                nc.sync.dma_start(hh[l][dr][100:101, :], ones_4s[:])
        c_ping = [[ppers.tile([100, 2], F32, tag=f"c_ping_{l}_{dr}",
                              name=f"c_ping_{l}_{dr}") for dr in range(2)]
                  for l in range(2)]
        c_pong = [[ppers.tile([100, 2], F32, tag=f"c_pong_{l}_{dr}",
                              name=f"c_pong_{l}_{dr}") for dr in range(2)]
                  for l in range(2)]

        def xw_layer(l):
            wih = wih0 if l == 0 else wih1
            for dr in range(2):
                for b in range(BL):
                    c = 2 * dr + b
                    for t0, tw in S_HALVES:
                        for g in range(4):
                            ps = pp.tile([128, 512], F32, tag="ps512")
                            for kc in range(2):
                                if l == 0:
                                    src = (xconv if dr == 0 else xconv_rev)[b][kc][:]
                                    rhs = src[:, t0:t0 + tw]
                                else:
                                    if dr == 0 and kc == 0:
                                        rhs = strided(hh[0][0][0:100, :], 1,
                                                      2 * t0 + b, 2, tw)
                                    elif dr == 0 and kc == 1:
                                        rhs = strided(hhrev[0][1][:], 1,
                                                      2 * t0 + b, 2, tw)
                                    elif dr == 1 and kc == 0:
                                        rhs = strided(hhrev[0][0][:], 1,
                                                      2 * t0 + b, 2, tw)
                                    else:
                                        rhs = strided(hh[0][1][0:100, :], 1,
                                                      2 * t0 + b, 2, tw)
                                nc.tensor.matmul(
                                    ps[0:100, 0:tw],
                                    wih[:, (dr * 2 + kc) * 400 + g * 100:
                                           (dr * 2 + kc) * 400 + g * 100 + 100],
                                    rhs, start=(kc == 0), stop=(kc == 1))
                            dst = strided(xg[:], 1, 16 * t0 + 4 * g + c, 16, tw)
                            nc.scalar.activation(
                                dst, ps[0:100, 0:tw], AF.Identity,
                                bias=blstm[:, (l * 2 + dr) * 4 + g:
                                           (l * 2 + dr) * 4 + g + 1])

        def recurrence(l):
            # two independent chains (dir 0 / dir 1), stage-interleaved so the
            # engines pipeline them against each other.
            # Q := C/4 ; tanh(c) = tanh(2Q)
            # v4 = (sig_G - 0.5)*sig_i ; Q = sig_f*Q + v4 ; h = sig_o*tanh(2Q)
            for k in range(KSTEPS):
                gps, sig, v4, th = {}, {}, {}, {}
                qo, qn = {}, {}
                for dr in range(2):
                    gps[dr] = ppg.tile([128, 8], F32, tag=f"gates_ps_{dr}",
                                       name=f"gps_{dr}", bufs=1)
                    # xW injection: one matmul, rhs 3D [100, 4(g), 2(c)]
                    xgs = strided(xg[:], 1, 16 * k + 2 * dr, 4, 4)
                    xgs = dataclasses.replace(
                        xgs, ap=[xgs.ap[0], [4, 4], [1, 2]])
                    g0 = gps[dr][0:100, :]
                    gout = dataclasses.replace(
                        g0, ap=[list(g0.ap[0]), [2, 4], [1, 2]])
                    nc.tensor.matmul(gout, ident[:], xgs,
                                     start=True, stop=True)
                    if k > 0:
                        for g in range(4):
                            nc.tensor.matmul(
                                gps[dr][:, 2 * g: 2 * g + 2],
                                whh[:, (l * 8 + g * 2 + dr) * 128:
                                       (l * 8 + g * 2 + dr) * 128 + 128],
                                hh[l][dr][0:100, 2 * (k - 1): 2 * k],
                                start=False, stop=True)
                for dr in range(2):
                    sig[dr] = psc.tile([100, 8], F32, tag=f"sig_{dr}",
                                       name=f"sig_{dr}")
                    nc.scalar.activation(sig[dr][:], gps[dr][0:100, :],
                                         AF.Sigmoid)
                for dr in range(2):
                    v4[dr] = psc.tile([100, 2], F32, tag=f"v4_{dr}",
                                      name=f"v4_{dr}")
                    nc.vector.scalar_tensor_tensor(
                        v4[dr][:], sig[dr][:, 4:6], 0.5, sig[dr][:, 0:2],
                        op0=ALU.subtract, op1=ALU.mult)
                    qo[dr], qn[dr] = ((c_ping[l][dr], c_pong[l][dr])
                                      if k % 2 == 0
                                      else (c_pong[l][dr], c_ping[l][dr]))
                    if k > 0:
                        t1 = psc.tile([100, 2], F32, tag=f"t1_{dr}",
                                      name=f"t1_{dr}")
                        nc.vector.tensor_tensor(t1[:], sig[dr][:, 2:4],
                                                qo[dr][:], op=ALU.mult)
                        nc.vector.tensor_tensor(qn[dr][:], t1[:], v4[dr][:],
                                                op=ALU.add)
                    else:
                        nc.vector.tensor_copy(qn[dr][:], v4[dr][:])
                for dr in range(2):
                    th[dr] = psc.tile([100, 2], F32, tag=f"th_{dr}",
                                      name=f"th_{dr}")
                    nc.scalar.activation(th[dr][:], qn[dr][:], AF.Tanh,
                                         scale=2.0)
                for dr in range(2):
                    nc.vector.tensor_tensor(
                        hh[l][dr][0:100, 2 * k:2 * k + 2],
                        sig[dr][:, 6:8], th[dr][:], op=ALU.mult)

        def reverse_hh(l):
            # hhrev[l][dr][:, 2k+b] = hh[l][dr][:, 2(S-1-k)+b]
            for dr in range(2):
                dstv = hhrev[l][dr][:].rearrange("p (k c) -> p k c", c=2)
                srcv = strided(
                    hh[l][dr][0:100, :].rearrange("p (k c) -> p k c", c=2),
                    1, S - 1, -1, S)
                nc.sync.dma_start(dstv, srcv)

        xw_layer(0)
        if os.environ.get("PHASE") == "xw0":
            return
        recurrence(0)
        reverse_hh(0)
        xw_layer(1)
        recurrence(1)
        reverse_hh(1)

        # ---------------- attention + mh + r ----------------

        def h1_ap(kc, b, t0, tw, with_ones=False):
            if kc == 0:
                hsrc = hh[1][0][0:101, :] if with_ones else hh[1][0][0:100, :]
                return strided(hsrc, 1, 2 * t0 + b, 2, tw)
            return strided(hhrev[1][1][:], 1, 2 * t0 + b, 2, tw)

        r_sb = ppers.tile([100, BL], F32, tag="r_sb")
        nc.vector.memset(r_sb[:], 0.0)

        for b in range(BL if not os.environ.get('SKIP_ATTN') else 0):
            # q/k head-pair tiles [64, S] f32r ; hp = h//2, rows 32*(h%2)
            qk_sb = {}
            for qk in range(2):
                for hp in range(4):
                    sb = pvt.tile([64, S], BF16, tag=f"qkx_{qk * 4 + hp}", name=f"qksb_{qk}_{hp}")
                    qk_sb[(qk, hp)] = sb
                    for s0, sw in S_HALVES:
                        ps = pp.tile([128, 512], F32, tag="ps512")
                        for kc in range(2):
                            nc.tensor.matmul(
                                ps[0:64, 0:sw],
                                wqk[:, ((qk * 2 + kc) * 4 + hp) * 64:
                                       ((qk * 2 + kc) * 4 + hp) * 64 + 64],
                                h1_ap(kc, b, s0, sw),
                                start=(kc == 0), stop=(kc == 1))
                        nc.scalar.activation(
                            sb[:, s0:s0 + sw], ps[0:64, 0:sw], AF.Identity,
                            bias=bqk[:, qk * 4 + hp: qk * 4 + hp + 1])
            # v natural [t, 8 heads x (32+ones+pad)] : [128, 272] per t-chunk
            v_sb = []
            for ti, (t0, tw) in enumerate(T_CHUNKS):
                vt = pvt.tile([128, 256], BF16, tag=f"v_sb_{ti}", name=f"vsb_{ti}")
                for hg in range(2):
                    ps = pp.tile([128, 512], F32, tag="ps512")
                    for kc in range(2):
                        nc.tensor.matmul(
                            ps[0:tw, 0:128],
                            h1_ap(kc, b, t0, tw, with_ones=(kc == 0)),
                            wv[0:(101 if kc == 0 else 100),
                               (kc * 2 + hg) * 128:(kc * 2 + hg) * 128 + 128],
                            start=(kc == 0), stop=(kc == 1))
                    nc.scalar.copy(vt[0:tw, hg * 128:hg * 128 + 128],
                                   ps[0:tw, 0:128])
                v_sb.append(vt)

            attn = [pvt.tile([32, S], F32, tag=f"attn_{h}", name=f"attn_{h}")
                    for h in range(NH)]
            # ---- pass 1 (per head): scores[t,s] -> exp -> unnormalized AV ----
            for h in range(NH):
                hp, sub = h // 2, h % 2
                at_ps = [ppg.tile([32, 512], F32, tag="gates_ps_0",
                                  name="at_ps0", bufs=1),
                         ppg.tile([32, 317], F32, tag="gates_ps_1",
                                  name="at_ps1", bufs=1)]
                for ti, (t0, tw) in enumerate(T_CHUNKS):
                    et = pe_pool.tile([128, S], BF16, tag="etile")
                    for si, (s0, sw) in enumerate([(0, 512), (512, 317)]):
                        sps = pp.tile([128, 512], F32, tag="ps512")
                        nc.tensor.matmul(
                            sps[0:tw, 0:sw],
                            qk_sb[(1, hp)][32 * sub:32 * sub + 32, t0:t0 + tw],
                            qk_sb[(0, hp)][32 * sub:32 * sub + 32, s0:s0 + sw],
                            start=True, stop=True)
                        nc.scalar.activation(et[0:tw, s0:s0 + sw],
                                             sps[0:tw, 0:sw], AF.Exp)
                        nc.tensor.matmul(
                            at_ps[si][:, 0:sw],
                            v_sb[ti][0:tw, 32 * h:32 * h + 32],
                            et[0:tw, s0:s0 + sw],
                            start=(ti == 0), stop=(ti == len(T_CHUNKS) - 1))
                for si, (s0, sw) in enumerate([(0, 512), (512, 317)]):
                    nc.scalar.activation(attn[h][:, s0:s0 + sw],
                                         at_ps[si][:, 0:sw], AF.Relu)

            # ---- pass 2 (per s-chunk): scores[s,t] -> softmax -> output ----
            for ci, (s0, sw) in enumerate(T_CHUNKS):
                invzc = pst.tile([128, 8], F32, tag="invzc")
                h1ab = [pst.tile([128, MH], F32, tag="h1a_a", name="h1a_a"),
                        pst.tile([128, MH], F32, tag="h1a_b", name="h1a_b")]
                for h in range(NH):
                    hp, sub = h // 2, h % 2
                    stp = ppg.tile([128, 832], F32, tag="ps_big", bufs=2)
                    for tj, (t0, tw) in enumerate([(0, 512), (512, 317)]):
                        nc.tensor.matmul(
                            stp[0:sw, t0:t0 + tw],
                            qk_sb[(0, hp)][32 * sub:32 * sub + 32, s0:s0 + sw],
                            qk_sb[(1, hp)][32 * sub:32 * sub + 32, t0:t0 + tw],
                            start=True, stop=True)
                    pex = pst.tile([128, 832], BF16, tag="pex")
                    zc = pst.tile([128, 1], F32, tag="zc")
                    nc.scalar.activation(pex[0:sw, 0:S], stp[0:sw, 0:S], AF.Exp,
                                         accum_out=zc[0:sw, :])
                    nc.vector.reciprocal(invzc[0:sw, h:h + 1], zc[0:sw, :])
                    stg = pst.tile([128, 832], F32, tag="stg")
                    nc.vector.tensor_scalar(stg[0:sw, 0:S], pex[0:sw, 0:S],
                                            invzc[0:sw, h:h + 1], None,
                                            op0=ALU.mult)
                    nc.sync.dma_start(
                        t["pattn"].ap()[b][s0:s0 + sw, h * S:h * S + S],
                        stg[0:sw, 0:S])
                    # mh: y_h = attn_h.T @ wmh_h ; h1a (+)= y_h * invz_h
                    yps = pp.tile([128, 512], F32, tag="ps512")
                    nc.tensor.matmul(yps[0:sw, 0:MH],
                                     attn[h][:, s0:s0 + sw],
                                     wmh[:, h * MH:(h + 1) * MH],
                                     start=True, stop=True)
                    nc.vector.scalar_tensor_tensor(
                        h1ab[h % 2][0:sw, :], yps[0:sw, 0:MH],
                        invzc[0:sw, h:h + 1],
                        bmhrep[0:sw, :] if h == 0 else h1ab[(h + 1) % 2][0:sw, :],
                        op0=ALU.mult, op1=ALU.add)
                h1r = psc.tile([128, MH], F32, tag="h1r")
                nc.scalar.activation(h1r[0:sw, :], h1ab[(NH - 1) % 2][0:sw, :],
                                     AF.Relu)
                rps = pp.tile([128, 512], F32, tag="ps512")
                nc.tensor.matmul(rps[0:100, 0:1], h1r[0:sw, :],
                                 ones_col_f[0:sw, :], start=True, stop=True)
                if b == 0 and ci == 0:
                    nc.vector.tensor_copy(r_sb[:, b:b + 1], rps[0:100, 0:1])
                else:
                    nc.vector.tensor_tensor(r_sb[:, b:b + 1], rps[0:100, 0:1],
                                            r_sb[:, b:b + 1], op=ALU.add)

        # ---------------- readout ----------------
        rcat = ppers.tile([100, 4], F32, tag="rcat")
        nc.scalar.copy(rcat[:, 0:2], r_sb[:])
        nc.scalar.activation(rcat[:, 2:4], rcat[:, 0:2], AF.Square)
        y_ps = pp.tile([2, 2], F32, tag="ps512", name="y_ps")
        nc.tensor.matmul(y_ps[:], rcat[:, 0:2], wfc[:], start=True, stop=True)
        sums_ps = pp.tile([1, 4], F32, tag="ps512", name="sums_ps")
        nc.tensor.matmul(sums_ps[:], ones_col_f[0:100, :], rcat[:],
                         start=True, stop=True)
        cc_in = ppers.tile([1, 4], F32, tag="cc_in")
        nc.scalar.copy(cc_in[:], sums_ps[:])
        cc_in_d = pdram.tile([1, 4], F32, name="cc_in_d")
        cc_out_d = pdram.tile([1, 4], F32, name="cc_out_d")
        nc.sync.dma_start(cc_in_d[:], cc_in[:])
        if getattr(nc, "_single_core_prof", False):
            nc.sync.dma_start(cc_out_d[:], cc_in_d[:])
        else:
            nc.gpsimd.collective_compute(
                "AllReduce", ALU.add,
                replica_groups=[list(range(N_CORES))],
                ins=[cc_in_d.opt()], outs=[cc_out_d.opt()])
        ar = ppers.tile([1, 4], F32, tag="ar")
        nc.sync.dma_start(ar[:], cc_out_d[:])

        # stats
        st = ppers.tile([1, 12], F32, tag="st")
        # cols: 0 S1, 1 S2, 2 mean, 3 S1^2, 4 S2/(n-1), 5 var, 6 sd, 7 rstd,
        #       8:10 ct, 11 tt=mean*rstd
        nc.vector.tensor_tensor(st[:, 0:2], strided(ar[:], 1, 0, 2, 2),
                                strided(ar[:], 1, 1, 2, 2), op=ALU.add)
        n = float(B * MH)
        nc.scalar.mul(st[:, 2:3], st[:, 0:1], 1.0 / n)
        nc.scalar.activation(st[:, 3:4], st[:, 0:1], AF.Square)
        nc.scalar.mul(st[:, 4:5], st[:, 1:2], 1.0 / (n - 1))
        nc.vector.scalar_tensor_tensor(
            st[:, 5:6], st[:, 3:4], -1.0 / n / (n - 1),
            st[:, 4:5], op0=ALU.mult, op1=ALU.add)
        nc.scalar.sqrt(st[:, 6:7], st[:, 5:6])
        nc.vector.reciprocal(st[:, 7:8], st[:, 6:7])
        nc.vector.tensor_tensor(st[:, 11:12], st[:, 2:3], st[:, 7:8], op=ALU.mult)
        nc.vector.scalar_tensor_tensor(st[:, 8:10], cfc[:, 0:2], st[:, 11:12],
                                       cfc[:, 2:4], op0=ALU.mult, op1=ALU.add)
        rep_ps = pp.tile([2, 3], F32, tag="ps512", name="rep_ps")
        nc.tensor.matmul(rep_ps[:], ones_row_f[:], st[:, 7:10],
                         start=True, stop=True)
        rep = ppers.tile([2, 3], F32, tag="rep")
        nc.scalar.copy(rep[:], rep_ps[:])
        z = ppers.tile([2, 2], F32, tag="z")
        nc.vector.scalar_tensor_tensor(z[:], y_ps[:], rep[:, 0:1], rep[:, 1:3],
                                       op0=ALU.mult, op1=ALU.add)
        pexp = ppers.tile([2, 3], F32, tag="pexp")
        nc.scalar.activation(pexp[:, 0:2], z[:], AF.Exp,
                             accum_out=pexp[:, 2:3])
        pinv = ppers.tile([2, 1], F32, tag="pinv")
        nc.vector.reciprocal(pinv[:], pexp[:, 2:3])
        pr = ppers.tile([2, 2], F32, tag="pr")
        nc.vector.tensor_scalar(pr[:], pexp[:, 0:2], pinv[:], None,
                                op0=ALU.mult)
        nc.sync.dma_start(t["probs"].ap(), pr[:])


# ============================================================ host wrapper
def _prep_weights(inp):
    f32 = np.float32
    bf16 = ml_dtypes.bfloat16
    w = {}

    # conv: fold BN. y = conv(x)*inv + (beta - mean*inv)
    wconv = np.zeros((2, 52, NK), f32)
    bconv = np.zeros((2, 2, H), f32)
    for bi, (cw, gm, bt, mn, vr) in enumerate([
            (inp['conv_p_w'], inp['bn_p_gamma'], inp['bn_p_beta'],
             inp['bn_p_mean'], inp['bn_p_var']),
            (inp['conv_e_w'], inp['bn_e_gamma'], inp['bn_e_beta'],
             inp['bn_e_mean'], inp['bn_e_var'])]):
        cw = np.asarray(cw, f32)
        inv = np.asarray(gm, f32) / np.sqrt(np.asarray(vr, f32) + BN_EPS)
        wconv[bi] = (cw * inv[:, None, None]).transpose(1, 2, 0).reshape(52, NK)
        bconv[bi] = (np.asarray(bt, f32) - np.asarray(mn, f32) * inv).reshape(2, H)
    w['wconv'] = wconv.transpose(1, 0, 2).reshape(52, 2 * NK)
    w['bconv'] = bconv.reshape(4, H).T.copy()

    # lstm (double g gate rows 200:300 for the sigmoid-only trick)
    wih_all = np.asarray(inp['lstm_w_ih'], f32).copy()
    whh_all = np.asarray(inp['lstm_w_hh'], f32).copy()
    bias_all = (np.asarray(inp['lstm_b_ih'], f32)
                + np.asarray(inp['lstm_b_hh'], f32)).copy()
    wih_all[:, :, 200:300] *= 2.0
    whh_all[:, :, 200:300] *= 2.0
    bias_all[:, :, 200:300] *= 2.0

    def pack_wih(l):
        out = np.zeros((2, 2, 4, 100, 100), f32)
        for d in range(2):
            for kc in range(2):
                for g in range(4):
                    out[d, kc, g] = wih_all[l, d, g * 100:(g + 1) * 100,
                                            kc * 100:(kc + 1) * 100].T
        return out

    w['wih0'] = pack_wih(0).transpose(3, 0, 1, 2, 4).reshape(100, 1600)
    w['wih1'] = pack_wih(1).transpose(3, 0, 1, 2, 4).reshape(100, 1600).astype(bf16)
    w['blstm'] = bias_all.reshape(16, 100).T.copy()
    whh_pack = np.zeros((2, 8, 100, 128), f32)
    for l in range(2):
        for g in range(4):
            for d in range(2):
                whh_pack[l, g * 2 + d, :, 0:100] = \
                    whh_all[l, d, g * 100:(g + 1) * 100].T
    w['whh'] = whh_pack.transpose(2, 0, 1, 3).reshape(100, 2048).astype(bf16)

    # attention
    scale = f32(1.0 / np.sqrt(HD))
    wq = np.asarray(inp['wq'], f32) * scale
    bq = np.asarray(inp['bq'], f32) * scale
    wk = np.asarray(inp['wk'], f32)
    bk = np.asarray(inp['bk'], f32)
    wv_ = np.asarray(inp['wv'], f32)
    bv = np.asarray(inp['bv'], f32)
    wqk = np.zeros((2, 2, 100, 4, 64), f32)
    bqk = np.zeros((2, 4, 64), f32)
    for qi, (wm, bm) in enumerate([(wq, bq), (wk, bk)]):
        for kc in range(2):
            for hp in range(4):
                for sub in range(2):
                    h = hp * 2 + sub
                    wqk[qi, kc, :, hp, sub * 32:(sub + 1) * 32] = \
                        wm[h, kc * 100:(kc + 1) * 100, :]
        for hp in range(4):
            bqk[qi, hp] = bm[hp * 2:(hp + 1) * 2].reshape(64)
    w['wqk'] = wqk.transpose(2, 0, 1, 3, 4).reshape(100, 1024).astype(bf16)
    w['bqk'] = bqk.reshape(8, 64).T.copy()
    wv_pack = np.zeros((2, 101, 2, 128), f32)
    for kc in range(2):
        for hg in range(2):
            for hl in range(4):
                h = hg * 4 + hl
                wv_pack[kc, 0:100, hg, hl * 32:(hl + 1) * 32] = \
                    wv_[h, kc * 100:(kc + 1) * 100, :]
                if kc == 0:
                    wv_pack[kc, 100, hg, hl * 32:(hl + 1) * 32] = bv[h]
    w['wv'] = wv_pack.transpose(1, 0, 2, 3).reshape(101, 512).astype(bf16)

    mh_w = np.asarray(inp['mh_w'], f32)
    wmh = np.zeros((32, NH * MH), f32)
    for h in range(NH):
        wmh[:, h * MH:(h + 1) * MH] = mh_w[h * 32:(h + 1) * 32, :]
    w['wmh'] = wmh
    w['bmh'] = np.asarray(inp['mh_b'], f32).reshape(1, MH)
    w['wfc'] = np.asarray(inp['fc3_w'], f32)
    cfc = np.zeros((1, 4), f32)
    cfc[0, 0:2] = -np.asarray(inp['fc3_w'], f32).sum(0)
    cfc[0, 2:4] = np.asarray(inp['fc3_b'], f32)
    w['cfc'] = cfc
    w['ident'] = np.eye(100, dtype=np.float32).astype(bf16)
    return w


_CACHE = {}


def kernel(**inputs):
    if "nc" not in _CACHE:
        _CACHE["nc"] = build_nc()
    nc = _CACHE["nc"]

    w = _prep_weights(inputs)
    xp = np.asarray(inputs['input_p'], np.float32)
    xe = np.asarray(inputs['input_e'], np.float32)

    in_maps = []
    for c in range(N_CORES):
        m = {k: np.ascontiguousarray(v) for k, v in w.items()}
        m['xp'] = np.ascontiguousarray(xp[c * BL:(c + 1) * BL])
        m['xe'] = np.ascontiguousarray(xe[c * BL:(c + 1) * BL])
        in_maps.append(m)

    res = run_bass_kernel_spmd(nc, in_maps, core_ids=list(range(N_CORES)),
                               **_CACHE.get("run_kwargs", {}))
    _CACHE["last_results"] = res

    probs = np.concatenate([res.results[c]["probs"] for c in range(N_CORES)], 0)
    pattn = np.concatenate([res.results[c]["pattn"] for c in range(N_CORES)], 0)
    return probs, pattn


if __name__ == "__main__":
    build_nc()
    print("build ok")
